# revision 1
# baseline (speedup 1.0000x reference)
"""Trainium2 Bass kernel for nn_CP_Based (CP-decomposition feature-product layer).

Math: out[b,u] = sum_r prod_f ( x0[b,f]*K[0,r,f,u] + x1[b,f]*K[1,r,f,u] )
  with x0 = 1/sqrt(1+X^2), x1 = X/sqrt(1+X^2).
Factor the normalization out of the f-product:
  out[b,u] = S[b] * sum_r prod_f ( K0[f,ru] + X[b,f]*K1[f,ru] ),
  S[b] = 1/sqrt(prod_f (1+X[b,f]^2)).
The 32-feature product is decomposed into 8 groups of 4 features. Each group's
product is a linear map from the 16 multilinear monomials of its 4 features:
  G_g[b,ru] = sum_m Q_g[b,m] * C_g[m,ru]        (K=32 matmul on TensorE)
with C_g packed on the host from `kernel` (tiny; zero rows pad each group to
32 so every matmul slice is 32-partition aligned). Monomials Q are built
batched for 512 rows at a time on VectorE, transposed via TensorE into wide
PSUM tiles so the monomial index lands on the contraction axis, copied once
per macro to SBUF (ScalarE), then 8 matmuls produce G_g and a 7-multiply
elementwise chain forms prod_g G_g; an indicator matmul sums over rank.

Sharding: pure data-parallel over batch: 131072 rows -> 8 cores x 16384.
"""

import sys

import numpy as np

sys.path.insert(0, "/opt/trn_rl_repo")

import concourse.bacc as bacc  # noqa: E402
import concourse.mybir as mybir  # noqa: E402
from concourse.bass_utils import run_bass_kernel_spmd  # noqa: E402
from concourse.tile import TileContext  # noqa: E402

F32 = mybir.dt.float32
AF = mybir.ActivationFunctionType
OP = mybir.AluOpType
AX = mybir.AxisListType

B_FULL = 131072
N_CORES = 8
B_CORE = B_FULL // N_CORES  # 16384
F = 32
R, U = 10, 8
RU = R * U  # 80
NG = 8  # feature groups of 4
TILE_B = 128
CHUNK = 4  # b-subtiles per macro tile -> N=512 matmuls
MACRO_B = TILE_B * CHUNK  # 512
N_MACRO = B_CORE // MACRO_B  # 32
CG = CHUNK * NG  # 32 (chunk, group) pairs


def build_nc():
    nc = bacc.Bacc()
    # host pre-arranges X as [macro, partition, chunk, feature] so each
    # macro's load is one contiguous 64 KB DMA
    X = nc.dram_tensor(
        "X", [N_MACRO, TILE_B, CHUNK, F], F32, kind="ExternalInput"
    )
    C = nc.dram_tensor("C", [128, 2 * RU], F32, kind="ExternalInput")
    ident = nc.dram_tensor("ident", [128, 128], F32, kind="ExternalInput")
    rind = nc.dram_tensor("rind", [RU, U], F32, kind="ExternalInput")
    out = nc.dram_tensor(
        "out", [N_MACRO, U, MACRO_B], F32, kind="ExternalOutput"
    )

    with TileContext(nc) as tc:
        with (
            tc.tile_pool(name="const", bufs=1) as cpool,
            tc.tile_pool(name="xin", bufs=3) as xpool,
            tc.tile_pool(name="work", bufs=3) as wpool,
            tc.tile_pool(name="qts", bufs=4) as qpool,
            tc.tile_pool(name="ps_t", bufs=2, space="PSUM") as tps,
            tc.tile_pool(name="ps_g", bufs=1, space="PSUM") as gps,
            tc.tile_pool(name="ps_o", bufs=2, space="PSUM") as ops_,
        ):
            c_sb = [
                cpool.tile([64, 2 * RU], F32, tag=f"c{h}", name=f"c{h}")
                for h in range(2)
            ]
            id_sb = cpool.tile([128, 128], F32, tag="id")
            ri_sb = cpool.tile([RU, U], F32, tag="ri")
            for h in range(2):
                nc.sync.dma_start(out=c_sb[h][:], in_=C[64 * h : 64 * (h + 1), :])
            nc.sync.dma_start(out=id_sb[:], in_=ident[:, :])
            nc.sync.dma_start(out=ri_sb[:], in_=rind[:, :])

            for mi in range(N_MACRO):
                b0 = mi * MACRO_B
                # x for 4 chunks: [128 b, 4 c, 32 f]
                xm = xpool.tile([TILE_B, CHUNK, F], F32, tag="x")
                nc.gpsimd.dma_start(out=xm[:], in_=X[mi])

                # --- S = 1/sqrt(prod_f (1+x^2)) for all 4 chunks ---
                sq = wpool.tile([TILE_B, CHUNK, F], F32, tag="sq")
                s_p = wpool.tile([TILE_B, CHUNK], F32, tag="s_p")
                s_r = wpool.tile([TILE_B, CHUNK], F32, tag="s_r")
                s_t = wpool.tile([TILE_B, CHUNK], F32, tag="s_t")
                nc.vector.tensor_mul(sq[:], xm[:], xm[:])
                nc.vector.tensor_scalar_add(sq[:], sq[:], 1.0)
                nc.vector.tensor_reduce(s_p[:], sq[:], AX.X, OP.mult)
                nc.vector.reciprocal(s_r[:], s_p[:])
                nc.scalar.sqrt(s_t[:], s_r[:])

                # --- monomial halves, batched over (chunk, group) = cg ---
                # pab[128, cg, 4] = (1, Xa, Xb, XaXb); pcd[128, cg, 4]
                pab = wpool.tile([TILE_B, CG, 4], F32, tag="pab")
                pcd = wpool.tile([TILE_B, CG, 4], F32, tag="pcd")
                xg = xm[:].rearrange("p c (g j) -> p (c g) j", j=4)
                nc.vector.memset(pab[:, :, 0:1], 1.0)
                nc.vector.memset(pcd[:, :, 0:1], 1.0)
                nc.vector.tensor_copy(pab[:, :, 1:3], xg[:, :, 0:2])
                nc.vector.tensor_copy(pcd[:, :, 1:3], xg[:, :, 2:4])
                nc.vector.tensor_mul(pab[:, :, 3:4], xg[:, :, 0:1], xg[:, :, 1:2])
                nc.vector.tensor_mul(pcd[:, :, 3:4], xg[:, :, 2:3], xg[:, :, 3:4])
                # fold S_c into group 0 of each chunk
                for c in range(CHUNK):
                    nc.vector.tensor_scalar(
                        pcd[:, c * NG, 0:4],
                        pcd[:, c * NG, 0:4],
                        s_t[:, c : c + 1],
                        None,
                        OP.mult,
                    )

                # --- Q[b, cg, i, j] = pab x pcd (one op, 512 cols) ---
                q = wpool.tile([TILE_B, CG, 4, 4], F32, tag="q")
                pab_b = pab[:].unsqueeze(3).broadcast_to([TILE_B, CG, 4, 4])
                pcd_b = pcd[:].unsqueeze(2).broadcast_to([TILE_B, CG, 4, 4])
                nc.vector.tensor_tensor(q[:], pab_b, pcd_b, OP.mult)

                # --- transpose Q (one [128,128] per chunk) -> wide PSUM ---
                qf = q[:].rearrange("p cg i j -> p (cg i j)")  # [128, 2048]
                ps_a = tps.tile([128, MACRO_B], F32, tag="ps_a")
                for c in range(CHUNK):
                    cw = slice(c * TILE_B, (c + 1) * TILE_B)
                    nc.tensor.transpose(
                        ps_a[:, cw], qf[:, c * 128 : (c + 1) * 128], id_sb[:]
                    )

                # --- copy QT halves to SBUF (2 wide ScalarE copies) ---
                # qts[t] rows: groups 4t..4t+3, 16 monomial rows each
                qts = [
                    qpool.tile([64, MACRO_B], F32, tag=f"qt{h}", name=f"qt{h}")
                    for h in range(2)
                ]
                nc.scalar.copy(qts[0][:], ps_a[0:64, :])
                nc.scalar.copy(qts[1][:], ps_a[64:128, :])

                # --- 8 group matmuls (K=32) + product chain ---
                # even groups: PSUM->SBUF copy on ScalarE; odd groups:
                # DVE multiplies PSUM x SBUF; GPSIMD folds the SBUF tree.
                g_ps = [
                    gps.tile([RU, MACRO_B], F32, tag=f"g{i}", name=f"g{i}")
                    for i in range(2)
                ]
                a_sb = [
                    qpool.tile([RU, MACRO_B], F32, tag=f"a{i}", name=f"a{i}")
                    for i in range(4)
                ]
                t_sb = [
                    qpool.tile([RU, MACRO_B], F32, tag=f"t{i}", name=f"t{i}")
                    for i in range(4)
                ]
                u_sb = [
                    qpool.tile([RU, MACRO_B], F32, tag=f"u{i}", name=f"u{i}")
                    for i in range(2)
                ]
                prod = qpool.tile([RU, MACRO_B], F32, tag="prod")
                for g in range(NG):
                    h, k = g // 2, g % 2
                    qt = qts[g // 4]
                    go = 32 * ((g % 4) // 2)  # == 32*(h%2)
                    csb = c_sb[h // 2]
                    dst = g_ps[g % 2]
                    nc.tensor.matmul(
                        dst[:],
                        csb[go : go + 32, RU * k : RU * (k + 1)],
                        qt[go : go + 32, :],
                        start=True,
                        stop=True,
                    )
                    # even groups: evacuate PSUM on ScalarE; odd: DVE mult
                    if g % 2 == 0:
                        nc.scalar.copy(a_sb[g // 2][:], dst[:])
                    else:
                        nc.vector.tensor_mul(
                            t_sb[g // 2][:], a_sb[g // 2][:], dst[:]
                        )
                nc.vector.tensor_mul(u_sb[0][:], t_sb[0][:], t_sb[1][:])
                nc.gpsimd.tensor_mul(u_sb[1][:], t_sb[2][:], t_sb[3][:])
                nc.vector.tensor_mul(prod[:], u_sb[0][:], u_sb[1][:])

                # --- sum over rank: out[u, b] = rind.T @ prod ---
                o_ps = ops_.tile([U, MACRO_B], F32, tag="o_ps")
                nc.tensor.matmul(o_ps[:], ri_sb[:], prod[:], start=True, stop=True)
                o_sb = qpool.tile([U, MACRO_B], F32, tag="o_sb")
                nc.scalar.copy(o_sb[:], o_ps[:])
                nc.sync.dma_start(out=out[mi], in_=o_sb[:])
    nc.finalize()
    return nc


def _pack_weights(kernel: np.ndarray):
    K = kernel.astype(np.float32)  # [2, R, F, U]
    C = np.zeros((128, 2 * RU), np.float32)
    bits = [(0, 0), (1, 0), (0, 1), (1, 1)]
    for g in range(NG):
        h, k = g // 2, g % 2
        r0 = 64 * (h // 2) + 32 * (h % 2) + 16 * k
        c0 = RU * k
        fs = [4 * g, 4 * g + 1, 4 * g + 2, 4 * g + 3]
        for i, (ba, bb) in enumerate(bits):
            for j, (bc, bd) in enumerate(bits):
                coef = (
                    K[ba, :, fs[0], :]
                    * K[bb, :, fs[1], :]
                    * K[bc, :, fs[2], :]
                    * K[bd, :, fs[3], :]
                )  # [R, U]
                C[r0 + i * 4 + j, c0 : c0 + RU] = coef.reshape(RU)
    ident = np.eye(128, dtype=np.float32)
    rind = np.zeros((RU, U), np.float32)
    for r in range(R):
        for u in range(U):
            rind[r * U + u, u] = 1.0
    return C, ident, rind


_NC_CACHE = {}


def kernel(X: np.ndarray, kernel: np.ndarray) -> np.ndarray:
    if "nc" not in _NC_CACHE:
        _NC_CACHE["nc"] = build_nc()
    nc = _NC_CACHE["nc"]
    C, ident, rind = _pack_weights(kernel)
    X = np.ascontiguousarray(X, dtype=np.float32)
    # [core, macro, chunk, partition, F] -> [core, macro, partition, chunk, F]
    Xd = (
        X.reshape(N_CORES, N_MACRO, CHUNK, TILE_B, F)
        .transpose(0, 1, 3, 2, 4)
        .copy()
    )
    in_maps = []
    for c in range(N_CORES):
        in_maps.append(
            {
                "X": Xd[c],
                "C": C,
                "ident": ident,
                "rind": rind,
            }
        )
    res = run_bass_kernel_spmd(nc, in_maps, core_ids=list(range(N_CORES)))
    outs = []
    for c in range(N_CORES):
        o = res.results[c]["out"]  # [N_MACRO, U, MACRO_B]
        outs.append(o.transpose(0, 2, 1).reshape(B_CORE, U))
    return np.concatenate(outs, axis=0).astype(np.float32)


if __name__ == "__main__":
    rng = np.random.default_rng(0)
    X = rng.standard_normal((B_FULL, F), dtype=np.float32)
    K = (rng.standard_normal((2, R, F, U)) * 0.24).astype(np.float32)
    y = kernel(X, K)
    print(y.shape, y.dtype, np.abs(y).max())



# revision 9
# speedup vs baseline: 1.8981x; 1.8981x over previous
"""Trainium2 Bass kernel for nn_CP_Based (CP-decomposition feature-product layer).

Math: out[b,u] = sum_r prod_f ( x0[b,f]*K[0,r,f,u] + x1[b,f]*K[1,r,f,u] )
  with x0 = 1/sqrt(1+X^2), x1 = X/sqrt(1+X^2).
Factor the normalization out of the f-product:
  out[b,u] = S[b] * sum_r prod_f ( K0[f,ru] + X[b,f]*K1[f,ru] ),
  S[b] = 1/sqrt(prod_f (1+X[b,f]^2)).
The 32-feature product is decomposed into 8 groups of 4 features. Each group's
product is a linear map from the 16 multilinear monomials of its 4 features:
  G_g[b,ru] = sum_m Q_g[b,m] * C_g[m,ru]        (K=32 matmul on TensorE)
with C_g packed on the host from `kernel` (tiny; zero rows pad each group to
32 so every matmul slice is 32-partition aligned). Monomials Q are built
in-place per 512-row macro tile, transposed via TensorE (bf16 data path) into
wide PSUM tiles so the monomial index lands on the contraction axis, then 8
matmuls produce G_g in four PSUM banks; a pairwise multiply tree (split across
DVE/Pool/bf16) forms prod_g G_g and an indicator matmul sums over rank.

Sharding: pure data-parallel over batch: 131072 rows -> 8 cores x 16384.
"""

import sys

import numpy as np

sys.path.insert(0, "/opt/trn_rl_repo")

import concourse.bacc as bacc  # noqa: E402
import concourse.mybir as mybir  # noqa: E402
from concourse.bass_utils import run_bass_kernel_spmd  # noqa: E402
from concourse.tile import TileContext  # noqa: E402

F32 = mybir.dt.float32
BF16 = mybir.dt.bfloat16
AF = mybir.ActivationFunctionType
OP = mybir.AluOpType
AX = mybir.AxisListType

B_FULL = 131072
N_CORES = 8
B_CORE = B_FULL // N_CORES  # 16384
F = 32
R, U = 10, 8
RU = R * U  # 80
NG = 8  # feature groups of 4
TILE_B = 128
CHUNK = 4  # b-subtiles per macro tile -> N=512 matmuls
MACRO_B = TILE_B * CHUNK  # 512
N_MACRO = B_CORE // MACRO_B  # 32
N_M2 = N_MACRO // 2  # 16 two-macro groups (DMA batching)
CG = CHUNK * NG  # 32 (chunk, group) pairs


def build_nc():
    nc = bacc.Bacc()
    # host pre-arranges X as [m2, partition, 2*chunk, feature] so each
    # 2-macro load is one contiguous 128 KB DMA
    X = nc.dram_tensor(
        "X", [N_M2, TILE_B, 2 * CHUNK, F], F32, kind="ExternalInput"
    )
    C = nc.dram_tensor("C", [128, 2 * RU], F32, kind="ExternalInput")
    ident = nc.dram_tensor("ident", [128, 128], BF16, kind="ExternalInput")
    rind = nc.dram_tensor("rind", [RU, U], F32, kind="ExternalInput")
    out = nc.dram_tensor("out", [N_M2, U, 2 * MACRO_B], F32, kind="ExternalOutput")

    with TileContext(nc) as tc:
        with (
            tc.tile_pool(name="const", bufs=1) as cpool,
            tc.tile_pool(name="xin", bufs=3) as xpool,
            tc.tile_pool(name="work", bufs=3) as wpool,
            tc.tile_pool(name="qts", bufs=2) as qpool,
            tc.tile_pool(name="ps_t", bufs=2, space="PSUM") as tps,
            tc.tile_pool(name="ps_g", bufs=1, space="PSUM") as gps,
            tc.tile_pool(name="ps_o", bufs=2, space="PSUM") as ops_,
        ):
            c_sb = [
                cpool.tile([64, 2 * RU], F32, tag=f"c{h}", name=f"c{h}")
                for h in range(2)
            ]
            id_sb = cpool.tile([128, 128], BF16, tag="id")
            ri_sb = cpool.tile([RU, U], F32, tag="ri")
            for h in range(2):
                nc.sync.dma_start(out=c_sb[h][:], in_=C[64 * h : 64 * (h + 1), :])
            nc.sync.dma_start(out=id_sb[:], in_=ident[:, :])
            nc.sync.dma_start(out=ri_sb[:], in_=rind[:, :])

            for mi in range(N_MACRO):
                m2, hh = mi // 2, mi % 2
                if hh == 0:
                    x2 = xpool.tile([TILE_B, 2 * CHUNK, F], F32, tag="x")
                    nc.sync.dma_start(out=x2[:], in_=X[m2])
                xm = x2[:, 4 * hh : 4 * hh + 4, :]  # [128, 4, 32]

                # --- S = 1/sqrt(prod_f (1+x^2)) for all 4 chunks ---
                sq = wpool.tile([TILE_B, CHUNK, F], F32, tag="sq")
                s_p = wpool.tile([TILE_B, CHUNK], F32, tag="s_p")
                s_r = wpool.tile([TILE_B, CHUNK], F32, tag="s_r")
                s_t = wpool.tile([TILE_B, CHUNK], F32, tag="s_t")
                nc.scalar.square(sq[:], xm)
                nc.scalar.add(sq[:], sq[:], 1.0)
                nc.vector.tensor_reduce(s_p[:], sq[:], AX.X, OP.mult)
                nc.vector.reciprocal(s_r[:], s_p[:])
                nc.scalar.sqrt(s_t[:], s_r[:])

                # --- monomials built in-place in q[128, cg, 4, 4] ---
                # q[b, cg, i, j] = pab_i(b,cg) * pcd_j(b,cg), with S folded
                # into group 0 of each chunk (rows i>=1 via the pab column,
                # row i=0 explicitly).
                q = wpool.tile([TILE_B, CG, 4, 4], F32, tag="q")
                xg = xm.rearrange("p c (g j) -> p (c g) j", j=4)
                # pab column (j=0): [1, Xa, Xb, XaXb]
                nc.vector.memset(q[:, :, 0:1, 0:1], 1.0)
                nc.scalar.copy(q[:, :, 1:3, 0:1], xg[:, :, 0:2].unsqueeze(3))
                nc.vector.tensor_mul(
                    q[:, :, 3:4, 0:1],
                    xg[:, :, 0:1].unsqueeze(3),
                    xg[:, :, 1:2].unsqueeze(3),
                )
                # pcd row (i=0): [1, Xc, Xd, XcXd] -- j=0 entry is the memset 1
                nc.scalar.copy(q[:, :, 0:1, 1:3], xg[:, :, 2:4].unsqueeze(2))
                nc.vector.tensor_mul(
                    q[:, :, 0:1, 3:4],
                    xg[:, :, 2:3].unsqueeze(2),
                    xg[:, :, 3:4].unsqueeze(2),
                )
                # fold S into group-0 pab column (rows i>=1) before the outer
                # product so the i>=1,j>=1 block inherits exactly one S
                q5 = q[:].rearrange("p (c g) i j -> p c g i j", g=NG)
                g0col = q5[:, :, 0, 1:4, 0:1]
                s_b1 = s_t[:].unsqueeze(2).unsqueeze(3).broadcast_to(
                    [TILE_B, CHUNK, 3, 1]
                )
                nc.vector.tensor_tensor(g0col, g0col, s_b1, OP.mult)
                # outer product fills i>=1, j>=1
                nc.vector.tensor_tensor(
                    q[:, :, 1:4, 1:4],
                    q[:, :, 1:4, 0:1].broadcast_to([TILE_B, CG, 3, 3]),
                    q[:, :, 0:1, 1:4].broadcast_to([TILE_B, CG, 3, 3]),
                    OP.mult,
                )
                # fold S into group-0 row i=0 (incl. the constant monomial)
                g0row = q5[:, :, 0, 0:1, 0:4]
                s_b2 = s_t[:].unsqueeze(2).unsqueeze(3).broadcast_to(
                    [TILE_B, CHUNK, 1, 4]
                )
                nc.vector.tensor_tensor(g0row, g0row, s_b2, OP.mult)

                # --- transpose Q (one [128,128] per chunk) -> bf16 PSUM ---
                qf = q[:].rearrange("p cg i j -> p (cg i j)")  # [128, 512]
                ps_a = tps.tile([128, MACRO_B], F32, tag="ps_a")
                for c in range(CHUNK):
                    cw = slice(c * TILE_B, (c + 1) * TILE_B)
                    nc.tensor.transpose(
                        ps_a[:, cw].bitcast(mybir.dt.float32r),
                        qf[:, c * 128 : (c + 1) * 128].bitcast(mybir.dt.float32r),
                        id_sb[:],
                    )

                # --- copy QT halves to SBUF (bf16) ---
                qts = [
                    qpool.tile([64, MACRO_B], BF16, tag=f"qt{h}", name=f"qt{h}")
                    for h in range(2)
                ]
                nc.scalar.copy(qts[0][:], ps_a[0:64, :])
                nc.scalar.copy(qts[1][:], ps_a[64:128, :])

                # --- 8 group matmuls (K=32) into 4 PSUM banks + mul tree ---
                g_ps = [
                    gps.tile([RU, MACRO_B], F32, tag=f"g{i}", name=f"g{i}")
                    for i in range(4)
                ]
                t_sb = qpool.tile([RU, 2, 2, MACRO_B], BF16, tag="t_sb")
                u_sb = qpool.tile([RU, 2, MACRO_B], BF16, tag="u_sb")
                prod = qpool.tile([RU, MACRO_B], BF16, tag="prod")
                for g in range(NG):
                    h, k = g // 2, g % 2
                    qt = qts[g // 4]
                    go = 32 * (h % 2)
                    csb = c_sb[h // 2]
                    dst = g_ps[g % 2 + 2 * ((g // 2) % 2)]
                    nc.tensor.matmul(
                        dst[:],
                        csb[go : go + 32, RU * k : RU * (k + 1)].bitcast(
                            mybir.dt.float32r
                        ),
                        qt[go : go + 32, :],
                        start=True,
                        stop=True,
                    )
                    if g % 2 == 1:
                        ti = g // 2  # 0..3
                        pair = ti % 2
                        eng = nc.vector if pair == 0 else nc.gpsimd
                        eng.tensor_tensor(
                            t_sb[:, ti // 2, ti % 2, :],
                            g_ps[2 * pair][:],
                            g_ps[2 * pair + 1][:],
                            OP.mult,
                        )
                # u = (t0*t1, t2*t3) in one wide bf16 op; prod = u0*u1
                nc.vector.tensor_tensor(
                    u_sb[:], t_sb[:, :, 0, :], t_sb[:, :, 1, :], OP.mult
                )
                nc.vector.tensor_tensor(
                    prod[:], u_sb[:, 0, :], u_sb[:, 1, :], OP.mult
                )

                # --- sum over rank: o_ps[u, b] = rind.T @ prod ---
                o_ps = ops_.tile([U, MACRO_B], F32, tag="o_ps")
                nc.tensor.matmul(
                    o_ps[:],
                    ri_sb[:].bitcast(mybir.dt.float32r),
                    prod[:],
                    start=True,
                    stop=True,
                )
                if hh == 0:
                    o2 = qpool.tile([U, 2, MACRO_B], F32, tag="o2")
                nc.scalar.copy(o2[:, hh, :], o_ps[:])
                if hh == 1:
                    nc.sync.dma_start(out=out[m2], in_=o2[:])
    nc.finalize()
    return nc


def _pack_weights(kernel: np.ndarray):
    K = kernel.astype(np.float32)  # [2, R, F, U]
    C = np.zeros((128, 2 * RU), np.float32)
    bits = [(0, 0), (1, 0), (0, 1), (1, 1)]
    for g in range(NG):
        h, k = g // 2, g % 2
        r0 = 64 * (h // 2) + 32 * (h % 2) + 16 * k
        c0 = RU * k
        fs = [4 * g, 4 * g + 1, 4 * g + 2, 4 * g + 3]
        for i, (ba, bb) in enumerate(bits):
            for j, (bc, bd) in enumerate(bits):
                coef = (
                    K[ba, :, fs[0], :]
                    * K[bb, :, fs[1], :]
                    * K[bc, :, fs[2], :]
                    * K[bd, :, fs[3], :]
                )  # [R, U]
                C[r0 + i * 4 + j, c0 : c0 + RU] = coef.reshape(RU)
    import ml_dtypes

    ident = np.eye(128, dtype=ml_dtypes.bfloat16)
    rind = np.zeros((RU, U), np.float32)
    for r in range(R):
        for u in range(U):
            rind[r * U + u, u] = 1.0
    return C, ident, rind


_NC_CACHE = {}


def kernel(X: np.ndarray, kernel: np.ndarray) -> np.ndarray:
    if "nc" not in _NC_CACHE:
        _NC_CACHE["nc"] = build_nc()
    nc = _NC_CACHE["nc"]
    C, ident, rind = _pack_weights(kernel)
    X = np.ascontiguousarray(X, dtype=np.float32)
    # [core, m2, half, chunk, partition, F] -> [core, m2, partition, half*chunk, F]
    Xd = (
        X.reshape(N_CORES, N_M2, 2 * CHUNK, TILE_B, F)
        .transpose(0, 1, 3, 2, 4)
        .copy()
    )
    in_maps = []
    for c in range(N_CORES):
        in_maps.append(
            {
                "X": Xd[c],
                "C": C,
                "ident": ident,
                "rind": rind,
            }
        )
    res = run_bass_kernel_spmd(nc, in_maps, core_ids=list(range(N_CORES)))
    outs = []
    for c in range(N_CORES):
        o = res.results[c]["out"]  # [N_M2, U, 2*MACRO_B]
        outs.append(o.transpose(0, 2, 1).reshape(B_CORE, U))
    return np.concatenate(outs, axis=0).astype(np.float32)


if __name__ == "__main__":
    rng = np.random.default_rng(0)
    X = rng.standard_normal((B_FULL, F), dtype=np.float32)
    K = (rng.standard_normal((2, R, F, U)) * 0.24).astype(np.float32)
    y = kernel(X, K)
    print(y.shape, y.dtype, np.abs(y).max())
